# revision 2
# baseline (speedup 1.0000x reference)
"""MCR2 variational loss on 8 Trainium2 NeuronCores.

Strategy (data-parallel over the sample axis n):
  - The heavy part of the loss is the per-class second-moment matrices
    M_j = Z^T diag(Pi_j) Z (plus the global gram Z^T Z), which reads all of
    Z/Pi once -> memory-bound. Everything downstream (logdet, log1p terms,
    Frobenius distance) is O(C*d^2) scalar work done on the host in fp64.
  - Fast path (Pi exactly one-hot): each sample contributes to exactly one
    class, so per-class partial grams over class-sorted rows give all M_j,
    and gram = sum_j M_j. Host sorts each core's row shard by class, pads
    each class segment to a 128-row multiple, and the device runs one fp32
    matmul per 128-row chunk, accumulating into that class's PSUM slice.
    Output per core: [128, C*128] partial M; host all-reduces in fp64.
  - Fallback (general dense Pi): host BLAS contraction.
"""

import numpy as np

EPS = 0.5
MU = 1.0
C = 10
N_TOTAL = 131072
D = 128
N_CORES = 8
N_PER_CORE = N_TOTAL // N_CORES
CHUNK = 128  # rows per matmul (PE contraction dim)
DMA_CHUNKS = 32  # chunks per DMA tile (2 MB)

_compiled_cache = {}


def _tile_sizes(total_chunks):
    sizes = []
    left = total_chunks
    while left > 0:
        t = min(DMA_CHUNKS, left)
        sizes.append(t)
        left -= t
    return sizes


def _build_bass_program(seg_chunks):
    """Build the SPMD bass program computing per-class partial grams.

    seg_chunks: list of C ints — 128-row chunks allotted to each class
    (identical on every core; zero-padded on the host). Device input "z" is
    the class-sorted, padded, PRE-TILED Z: for each DMA tile t of tsz
    chunks, a [128, tsz*128] block (SBUF partition p's data contiguous).
    Output "m_out": [128, C*128] partial M (d on partitions, (j,e) free).
    """
    import concourse.bacc as bacc
    import concourse.tile as tile
    from concourse import mybir
    from contextlib import ExitStack

    total_chunks = sum(seg_chunks)
    tile_sizes = _tile_sizes(total_chunks)

    chunk_cls = []
    for j, k in enumerate(seg_chunks):
        chunk_cls.extend([j] * k)
    first = {}
    last = {}
    for i, j in enumerate(chunk_cls):
        first.setdefault(j, i)
        last[j] = i

    nc = bacc.Bacc("TRN2", target_bir_lowering=False, debug=False, num_devices=N_CORES)
    z = nc.dram_tensor(
        "z", [total_chunks * CHUNK, D], mybir.dt.float32, kind="ExternalInput"
    ).ap()
    out = nc.dram_tensor(
        "m_out", [D, C * D], mybir.dt.float32, kind="ExternalOutput"
    ).ap()

    with tile.TileContext(nc) as tc:
        with ExitStack() as ctx:
            psum = ctx.enter_context(tc.tile_pool(name="psum", bufs=1, space="PSUM"))
            opool = ctx.enter_context(tc.tile_pool(name="o", bufs=1))
            acc = psum.tile([128, C * D], mybir.dt.float32)
            row0 = 0
            cidx = 0
            for t, tsz in enumerate(tile_sizes):
                pool = ctx.enter_context(tc.tile_pool(name=f"z{t}", bufs=1))
                tl = pool.tile([128, tsz * D], mybir.dt.float32)
                # rows [row0, row0+128*tsz) reinterpreted as [128, tsz*D]
                src = z[row0 : row0 + CHUNK * tsz, :].rearrange(
                    "(p k) d -> p (k d)", p=128
                )
                nc.sync.dma_start(tl[:], src)
                row0 += CHUNK * tsz
                for k in range(tsz):
                    j = chunk_cls[cidx]
                    chunk = tl[:, k * D : (k + 1) * D]
                    nc.tensor.matmul(
                        acc[:, j * D : (j + 1) * D],
                        chunk,
                        chunk,
                        start=(cidx == first[j]),
                        stop=(cidx == last[j]),
                        skip_group_check=True,
                    )
                    cidx += 1
            o = opool.tile([128, C * D], mybir.dt.float32)
            nc.vector.tensor_copy(o[:], acc[:])
            nc.sync.dma_start(out[:], o[:])
    nc.compile()
    return nc


def _is_one_hot(Pi):
    if not (Pi.sum(axis=1) == 1.0).all():
        return False
    if not (Pi.max(axis=1) == 1.0).all():
        return False
    return np.count_nonzero(Pi) == Pi.shape[0]


def _fast_path_M(Z, Pi):
    """Per-class second moments via the device. Returns M [C, D, D] fp64."""
    from concourse.bass_utils import run_bass_kernel_spmd

    labels = np.argmax(Pi, axis=1).astype(np.int64)

    shard_rows = []
    counts = np.zeros((N_CORES, C), dtype=np.int64)
    for c in range(N_CORES):
        lab = labels[c * N_PER_CORE : (c + 1) * N_PER_CORE]
        order = np.argsort(lab, kind="stable")
        shard_rows.append(order + c * N_PER_CORE)
        counts[c] = np.bincount(lab, minlength=C)

    seg_chunks = [max(1, int(np.ceil(counts[:, j].max() / CHUNK))) for j in range(C)]
    total_chunks = sum(seg_chunks)
    tile_sizes = _tile_sizes(total_chunks)

    key = tuple(seg_chunks)
    if key not in _compiled_cache:
        _compiled_cache[key] = _build_bass_program(seg_chunks)
    nc = _compiled_cache[key]

    offs = np.concatenate([[0], np.cumsum(seg_chunks)]) * CHUNK
    in_maps = []
    for c in range(N_CORES):
        zbuf = np.zeros((total_chunks * CHUNK, D), dtype=np.float32)
        rows = shard_rows[c]
        pos = 0
        for j in range(C):
            nj = counts[c, j]
            zbuf[offs[j] : offs[j] + nj] = Z[rows[pos : pos + nj]]
            pos += nj
        # pre-tile each DMA block: [tsz, 128, D] -> [128, tsz*D]
        parts = []
        start = 0
        for tsz in tile_sizes:
            blk = zbuf[start * CHUNK : (start + tsz) * CHUNK]
            parts.append(
                np.ascontiguousarray(
                    blk.reshape(tsz, CHUNK, D).transpose(1, 0, 2)
                ).reshape(-1)
            )
            start += tsz
        zdev = np.concatenate(parts).reshape(total_chunks * CHUNK, D)
        in_maps.append({"z": zdev})

    res = run_bass_kernel_spmd(nc, in_maps, list(range(N_CORES)))
    M = np.zeros((C, D, D), dtype=np.float64)
    for c in range(N_CORES):
        o = res.results[c]["m_out"].astype(np.float64)  # [D, C*D]
        M += o.reshape(D, C, D).transpose(1, 0, 2)
    return M


def _dense_path_M(Z, Pi):
    """General dense Pi: host BLAS contraction. Returns (M, gram) fp64."""
    Zf = np.ascontiguousarray(Z, dtype=np.float32)
    A = (Pi[:, :, None].astype(np.float32) * Zf[:, None, :]).reshape(Zf.shape[0], -1)
    M = (A.T @ Zf).reshape(C, D, D).astype(np.float64)
    gram = (Zf.T @ Zf).astype(np.float64)
    return M, gram


def kernel(Z, Pi, Us):
    Z = np.asarray(Z, dtype=np.float32)
    Pi = np.asarray(Pi, dtype=np.float32)
    Us = np.asarray(Us, dtype=np.float32)
    n, d = Z.shape

    if n == N_TOTAL and d == D and Pi.shape == (n, C) and _is_one_hot(Pi):
        M = _fast_path_M(Z, Pi)
        gram = M.sum(axis=0)
    else:
        M, gram = _dense_path_M(Z, Pi)

    nf = float(n)
    df = float(d)

    A = np.eye(d, dtype=np.float64) + (df / (nf * EPS)) * gram
    sign, logabsdet = np.linalg.slogdet(A)
    loss_R = 0.5 * logabsdet

    trPi = Pi.astype(np.float64).sum(axis=0)
    col_norms_sq = (Us.astype(np.float64) ** 2).sum(axis=1)  # [C, d]
    with np.errstate(divide="ignore"):
        per_class = np.log1p((df / (trPi[:, None] * EPS)) * col_norms_sq).sum(axis=1)
    loss_Rc = ((trPi / (2.0 * nf)) * per_class).sum()

    Us64 = Us.astype(np.float64)
    UUt = np.einsum("jdk,jek->jde", Us64, Us64)
    loss_reg = 0.5 * MU * ((M - UUt) ** 2).sum()

    loss_obj = loss_R - loss_Rc - loss_reg
    return (
        np.float32(-loss_obj),
        np.float32(loss_R),
        np.float32(loss_Rc),
        np.float32(loss_reg),
    )


# revision 3
# speedup vs baseline: 1.0344x; 1.0344x over previous
"""MCR2 variational loss on 8 Trainium2 NeuronCores.

Strategy (data-parallel over the sample axis n):
  - The heavy part of the loss is the per-class second-moment matrices
    M_j = Z^T diag(Pi_j) Z (plus the global gram Z^T Z), which reads all of
    Z/Pi once -> memory-bound. Everything downstream (logdet, log1p terms,
    Frobenius distance) is O(C*d^2) scalar work done on the host in fp64.
  - Fast path (Pi exactly one-hot): each sample contributes to exactly one
    class, so per-class partial grams over class-sorted rows give all M_j,
    and gram = sum_j M_j. Host distributes rows so every core gets an
    almost equal share of each class, pads each class segment to a 128-row
    multiple, and the device runs one fp32 matmul per 128-row chunk,
    accumulating into that class's PSUM slice. DMA tiles ramp up in size
    and alternate between the two HWDGE rings (SP + ACT) to start the PE
    early and sustain full HBM bandwidth. Output per core: [128, C*128]
    partial M, stored eagerly per class; host all-reduces in fp64.
  - Fallback (general dense Pi): host BLAS contraction.
"""

import numpy as np

EPS = 0.5
MU = 1.0
C = 10
N_TOTAL = 131072
D = 128
N_CORES = 8
CHUNK = 128  # rows per matmul (PE contraction dim)

_compiled_cache = {}


def _dma_tile_sizes(total_chunks):
    """Ramped tile sizes (in chunks): small first so the PE starts early."""
    sizes = []
    ramp = [2, 2, 4, 4, 8, 8, 16, 16]
    left = total_chunks
    for r in ramp:
        if left <= 0:
            break
        t = min(r, left)
        sizes.append(t)
        left -= t
    while left > 0:
        t = min(32, left)
        sizes.append(t)
        left -= t
    return sizes


def _build_bass_program(seg_chunks):
    """SPMD bass program computing per-class partial grams.

    seg_chunks: list of C ints — 128-row chunks per class (identical on all
    cores; zero padded on the host). Device input "z" is the class-sorted,
    padded, PRE-TILED Z: for each DMA tile t of tsz chunks, a contiguous
    [128, tsz*128] block (each SBUF partition's data contiguous in DRAM).
    Output "m_out": [128, C*128] partial M (d on partitions, (j,e) free).
    """
    import concourse.bacc as bacc
    import concourse.tile as tile
    from concourse import mybir
    from contextlib import ExitStack

    total_chunks = sum(seg_chunks)
    tile_sizes = _dma_tile_sizes(total_chunks)

    chunk_cls = []
    for j, k in enumerate(seg_chunks):
        chunk_cls.extend([j] * k)
    first = {}
    last = {}
    for i, j in enumerate(chunk_cls):
        first.setdefault(j, i)
        last[j] = i

    nc = bacc.Bacc("TRN2", target_bir_lowering=False, debug=False, num_devices=N_CORES)
    z = nc.dram_tensor(
        "z", [total_chunks * CHUNK, D], mybir.dt.float32, kind="ExternalInput"
    ).ap()
    out = nc.dram_tensor(
        "m_out", [D, C * D], mybir.dt.float32, kind="ExternalOutput"
    ).ap()

    with tile.TileContext(nc) as tc:
        with ExitStack() as ctx:
            psum = ctx.enter_context(tc.tile_pool(name="psum", bufs=1, space="PSUM"))
            opool = ctx.enter_context(tc.tile_pool(name="o", bufs=1))
            acc = psum.tile([128, C * D], mybir.dt.float32)
            sb_out = opool.tile([128, C * D], mybir.dt.float32)
            row0 = 0
            cidx = 0
            for t, tsz in enumerate(tile_sizes):
                pool = ctx.enter_context(tc.tile_pool(name=f"z{t}", bufs=1))
                tl = pool.tile([128, tsz * D], mybir.dt.float32)
                src = z[row0 : row0 + CHUNK * tsz, :].rearrange(
                    "(p k) d -> p (k d)", p=128
                )
                # alternate between the two HWDGE rings
                if t % 2 == 0:
                    nc.sync.dma_start(tl[:], src)
                else:
                    nc.scalar.dma_start(tl[:], src)
                row0 += CHUNK * tsz
                for k in range(tsz):
                    j = chunk_cls[cidx]
                    chunk = tl[:, k * D : (k + 1) * D]
                    nc.tensor.matmul(
                        acc[:, j * D : (j + 1) * D],
                        chunk,
                        chunk,
                        start=(cidx == first[j]),
                        stop=(cidx == last[j]),
                        skip_group_check=True,
                    )
                    if cidx == last[j]:
                        # eager per-class drain of the finished PSUM slice
                        sl = slice(j * D, (j + 1) * D)
                        nc.vector.tensor_copy(sb_out[:, sl], acc[:, sl])
                        nc.gpsimd.dma_start(out[:, sl], sb_out[:, sl])
                    cidx += 1
    nc.compile()
    return nc


def _is_one_hot(Pi):
    if not (Pi.sum(axis=1) == 1.0).all():
        return False
    if not (Pi.max(axis=1) == 1.0).all():
        return False
    return np.count_nonzero(Pi) == Pi.shape[0]


def _fast_path_M(Z, Pi):
    """Per-class second moments via the device. Returns M [C, D, D] fp64."""
    from concourse.bass_utils import run_bass_kernel_spmd

    labels = np.argmax(Pi, axis=1)

    # balance every class across cores: class j's rows are dealt out in
    # near-equal contiguous slices, so per-class per-core counts differ by
    # at most 1 and padding is minimal
    order = np.argsort(labels, kind="stable")
    cls_counts = np.bincount(labels, minlength=C)
    cls_offs = np.concatenate([[0], np.cumsum(cls_counts)])

    counts = np.zeros((N_CORES, C), dtype=np.int64)
    for j in range(C):
        m = cls_counts[j]
        base, rem = divmod(m, N_CORES)
        for c in range(N_CORES):
            counts[c, j] = base + (1 if c < rem else 0)

    seg_chunks = [max(1, int(np.ceil(counts[:, j].max() / CHUNK))) for j in range(C)]
    total_chunks = sum(seg_chunks)
    tile_sizes = _dma_tile_sizes(total_chunks)

    key = tuple(seg_chunks)
    if key not in _compiled_cache:
        _compiled_cache[key] = _build_bass_program(seg_chunks)
    nc = _compiled_cache[key]

    offs = np.concatenate([[0], np.cumsum(seg_chunks)]) * CHUNK
    in_maps = []
    for c in range(N_CORES):
        zbuf = np.zeros((total_chunks * CHUNK, D), dtype=np.float32)
        for j in range(C):
            lo = cls_offs[j] + counts[:c, j].sum()
            nj = counts[c, j]
            zbuf[offs[j] : offs[j] + nj] = Z[order[lo : lo + nj]]
        # pre-tile each DMA block: [tsz, 128, D] -> [128, tsz*D]
        parts = []
        start = 0
        for tsz in tile_sizes:
            blk = zbuf[start * CHUNK : (start + tsz) * CHUNK]
            parts.append(
                np.ascontiguousarray(
                    blk.reshape(tsz, CHUNK, D).transpose(1, 0, 2)
                ).reshape(-1)
            )
            start += tsz
        zdev = np.concatenate(parts).reshape(total_chunks * CHUNK, D)
        in_maps.append({"z": zdev})

    res = run_bass_kernel_spmd(nc, in_maps, list(range(N_CORES)))
    M = np.zeros((C, D, D), dtype=np.float64)
    for c in range(N_CORES):
        o = res.results[c]["m_out"].astype(np.float64)  # [D, C*D]
        M += o.reshape(D, C, D).transpose(1, 0, 2)
    return M


def _dense_path_M(Z, Pi):
    """General dense Pi: host BLAS contraction. Returns (M, gram) fp64."""
    Zf = np.ascontiguousarray(Z, dtype=np.float32)
    A = (Pi[:, :, None].astype(np.float32) * Zf[:, None, :]).reshape(Zf.shape[0], -1)
    M = (A.T @ Zf).reshape(C, D, D).astype(np.float64)
    gram = (Zf.T @ Zf).astype(np.float64)
    return M, gram


def kernel(Z, Pi, Us):
    Z = np.asarray(Z, dtype=np.float32)
    Pi = np.asarray(Pi, dtype=np.float32)
    Us = np.asarray(Us, dtype=np.float32)
    n, d = Z.shape

    if n == N_TOTAL and d == D and Pi.shape == (n, C) and _is_one_hot(Pi):
        M = _fast_path_M(Z, Pi)
        gram = M.sum(axis=0)
    else:
        M, gram = _dense_path_M(Z, Pi)

    nf = float(n)
    df = float(d)

    A = np.eye(d, dtype=np.float64) + (df / (nf * EPS)) * gram
    sign, logabsdet = np.linalg.slogdet(A)
    loss_R = 0.5 * logabsdet

    trPi = Pi.astype(np.float64).sum(axis=0)
    col_norms_sq = (Us.astype(np.float64) ** 2).sum(axis=1)  # [C, d]
    with np.errstate(divide="ignore"):
        per_class = np.log1p((df / (trPi[:, None] * EPS)) * col_norms_sq).sum(axis=1)
    loss_Rc = ((trPi / (2.0 * nf)) * per_class).sum()

    Us64 = Us.astype(np.float64)
    UUt = np.einsum("jdk,jek->jde", Us64, Us64)
    loss_reg = 0.5 * MU * ((M - UUt) ** 2).sum()

    loss_obj = loss_R - loss_Rc - loss_reg
    return (
        np.float32(-loss_obj),
        np.float32(loss_R),
        np.float32(loss_Rc),
        np.float32(loss_reg),
    )


# revision 5
# speedup vs baseline: 1.0465x; 1.0117x over previous
"""MCR2 variational loss on 8 Trainium2 NeuronCores.

Strategy (data-parallel over the sample axis n):
  - The heavy part of the loss is the per-class second-moment matrices
    M_j = Z^T diag(Pi_j) Z (plus the global gram Z^T Z), which reads all of
    Z/Pi once -> memory-bound. Everything downstream (logdet, log1p terms,
    Frobenius distance) is O(C*d^2) scalar work done on the host in fp64.
  - Fast path (Pi exactly one-hot): each sample contributes to exactly one
    class, so per-class partial grams over class-sorted rows give all M_j,
    and gram = sum_j M_j. Host distributes rows so every core gets an
    almost equal share of each class, pads each class segment to a 128-row
    multiple, and the device runs one fp32 matmul per 128-row chunk,
    accumulating into that class's PSUM slice. DMA tiles ramp up in size
    and alternate between the two HWDGE rings (SP + ACT) to start the PE
    early and sustain full HBM bandwidth. Output per core: [128, C*128]
    partial M, stored eagerly per class; host all-reduces in fp64.
  - Fallback (general dense Pi): host BLAS contraction.
"""

import numpy as np

EPS = 0.5
MU = 1.0
C = 10
N_TOTAL = 131072
D = 128
N_CORES = 8
CHUNK = 128  # rows per matmul (PE contraction dim)

_compiled_cache = {}


def _dma_tile_sizes(total_chunks):
    """Ramped tile sizes (in chunks): small first so the PE starts early."""
    sizes = []
    ramp = [4, 4, 8, 8, 16, 16]
    left = total_chunks
    for r in ramp:
        if left <= 0:
            break
        t = min(r, left)
        sizes.append(t)
        left -= t
    while left > 0:
        t = min(32, left)
        sizes.append(t)
        left -= t
    return sizes


def _build_bass_program(seg_chunks):
    """SPMD bass program computing per-class partial grams.

    seg_chunks: list of C ints — 128-row chunks per class (identical on all
    cores; zero padded on the host). Device input "z" is the class-sorted,
    padded, PRE-TILED Z: for each DMA tile t of tsz chunks, a contiguous
    [128, tsz*128] block (each SBUF partition's data contiguous in DRAM).
    Output "m_out": [128, C*128] partial M (d on partitions, (j,e) free).
    """
    import concourse.bacc as bacc
    import concourse.tile as tile
    from concourse import mybir
    from contextlib import ExitStack

    total_chunks = sum(seg_chunks)
    tile_sizes = _dma_tile_sizes(total_chunks)

    chunk_cls = []
    for j, k in enumerate(seg_chunks):
        chunk_cls.extend([j] * k)
    first = {}
    last = {}
    for i, j in enumerate(chunk_cls):
        first.setdefault(j, i)
        last[j] = i

    nc = bacc.Bacc("TRN2", target_bir_lowering=False, debug=False, num_devices=N_CORES)
    z = nc.dram_tensor(
        "z", [total_chunks * CHUNK, D], mybir.dt.float32, kind="ExternalInput"
    ).ap()
    out = nc.dram_tensor(
        "m_out", [D, C * D], mybir.dt.float32, kind="ExternalOutput"
    ).ap()

    with tile.TileContext(nc) as tc:
        with ExitStack() as ctx:
            psum = ctx.enter_context(tc.tile_pool(name="psum", bufs=1, space="PSUM"))
            opool = ctx.enter_context(tc.tile_pool(name="o", bufs=1))
            acc = psum.tile([128, C * D], mybir.dt.float32)
            sb_out = opool.tile([128, C * D], mybir.dt.float32)
            row0 = 0
            cidx = 0
            for t, tsz in enumerate(tile_sizes):
                pool = ctx.enter_context(tc.tile_pool(name=f"z{t}", bufs=1))
                tl = pool.tile([128, tsz * D], mybir.dt.float32)
                src = z[row0 : row0 + CHUNK * tsz, :].rearrange(
                    "(p k) d -> p (k d)", p=128
                )
                # alternate between the two HWDGE rings
                if t % 2 == 0:
                    nc.sync.dma_start(tl[:], src)
                else:
                    nc.scalar.dma_start(tl[:], src)
                row0 += CHUNK * tsz
                for k in range(tsz):
                    j = chunk_cls[cidx]
                    chunk = tl[:, k * D : (k + 1) * D]
                    nc.tensor.matmul(
                        acc[:, j * D : (j + 1) * D],
                        chunk,
                        chunk,
                        start=(cidx == first[j]),
                        stop=(cidx == last[j]),
                        skip_group_check=True,
                    )
                    # drain finished PSUM *banks* (4 classes = 512 f32 = one
                    # 2KB bank) so the DVE read never shares a bank with
                    # in-flight PE writes (which would serialize the PE)
                    if cidx == last[j] and (j % 4 == 3 or j == C - 1):
                        g0 = (j // 4) * 4
                        sl = slice(g0 * D, (j + 1) * D)
                        nc.vector.tensor_copy(sb_out[:, sl], acc[:, sl])
                        # stores ride the (by now mostly idle) HWDGE rings
                        if (j // 4) % 2 == 0:
                            nc.sync.dma_start(out[:, sl], sb_out[:, sl])
                        else:
                            nc.scalar.dma_start(out[:, sl], sb_out[:, sl])
                    cidx += 1
    nc.compile()
    return nc


def _is_one_hot(Pi):
    if not (Pi.sum(axis=1) == 1.0).all():
        return False
    if not (Pi.max(axis=1) == 1.0).all():
        return False
    return np.count_nonzero(Pi) == Pi.shape[0]


def _fast_path_M(Z, Pi):
    """Per-class second moments via the device. Returns M [C, D, D] fp64."""
    from concourse.bass_utils import run_bass_kernel_spmd

    labels = np.argmax(Pi, axis=1)

    # balance every class across cores: class j's rows are dealt out in
    # near-equal contiguous slices, so per-class per-core counts differ by
    # at most 1 and padding is minimal
    order = np.argsort(labels, kind="stable")
    cls_counts = np.bincount(labels, minlength=C)
    cls_offs = np.concatenate([[0], np.cumsum(cls_counts)])

    counts = np.zeros((N_CORES, C), dtype=np.int64)
    for j in range(C):
        m = cls_counts[j]
        base, rem = divmod(m, N_CORES)
        for c in range(N_CORES):
            counts[c, j] = base + (1 if c < rem else 0)

    seg_chunks = [max(1, int(np.ceil(counts[:, j].max() / CHUNK))) for j in range(C)]
    total_chunks = sum(seg_chunks)
    tile_sizes = _dma_tile_sizes(total_chunks)

    key = tuple(seg_chunks)
    if key not in _compiled_cache:
        _compiled_cache[key] = _build_bass_program(seg_chunks)
    nc = _compiled_cache[key]

    offs = np.concatenate([[0], np.cumsum(seg_chunks)]) * CHUNK
    in_maps = []
    for c in range(N_CORES):
        zbuf = np.zeros((total_chunks * CHUNK, D), dtype=np.float32)
        for j in range(C):
            lo = cls_offs[j] + counts[:c, j].sum()
            nj = counts[c, j]
            zbuf[offs[j] : offs[j] + nj] = Z[order[lo : lo + nj]]
        # pre-tile each DMA block: [tsz, 128, D] -> [128, tsz*D]
        parts = []
        start = 0
        for tsz in tile_sizes:
            blk = zbuf[start * CHUNK : (start + tsz) * CHUNK]
            parts.append(
                np.ascontiguousarray(
                    blk.reshape(tsz, CHUNK, D).transpose(1, 0, 2)
                ).reshape(-1)
            )
            start += tsz
        zdev = np.concatenate(parts).reshape(total_chunks * CHUNK, D)
        in_maps.append({"z": zdev})

    res = run_bass_kernel_spmd(nc, in_maps, list(range(N_CORES)))
    M = np.zeros((C, D, D), dtype=np.float64)
    for c in range(N_CORES):
        o = res.results[c]["m_out"].astype(np.float64)  # [D, C*D]
        M += o.reshape(D, C, D).transpose(1, 0, 2)
    return M


def _dense_path_M(Z, Pi):
    """General dense Pi: host BLAS contraction. Returns (M, gram) fp64."""
    Zf = np.ascontiguousarray(Z, dtype=np.float32)
    A = (Pi[:, :, None].astype(np.float32) * Zf[:, None, :]).reshape(Zf.shape[0], -1)
    M = (A.T @ Zf).reshape(C, D, D).astype(np.float64)
    gram = (Zf.T @ Zf).astype(np.float64)
    return M, gram


def kernel(Z, Pi, Us):
    Z = np.asarray(Z, dtype=np.float32)
    Pi = np.asarray(Pi, dtype=np.float32)
    Us = np.asarray(Us, dtype=np.float32)
    n, d = Z.shape

    if n == N_TOTAL and d == D and Pi.shape == (n, C) and _is_one_hot(Pi):
        M = _fast_path_M(Z, Pi)
        gram = M.sum(axis=0)
    else:
        M, gram = _dense_path_M(Z, Pi)

    nf = float(n)
    df = float(d)

    A = np.eye(d, dtype=np.float64) + (df / (nf * EPS)) * gram
    sign, logabsdet = np.linalg.slogdet(A)
    loss_R = 0.5 * logabsdet

    trPi = Pi.astype(np.float64).sum(axis=0)
    col_norms_sq = (Us.astype(np.float64) ** 2).sum(axis=1)  # [C, d]
    with np.errstate(divide="ignore"):
        per_class = np.log1p((df / (trPi[:, None] * EPS)) * col_norms_sq).sum(axis=1)
    loss_Rc = ((trPi / (2.0 * nf)) * per_class).sum()

    Us64 = Us.astype(np.float64)
    UUt = np.einsum("jdk,jek->jde", Us64, Us64)
    loss_reg = 0.5 * MU * ((M - UUt) ** 2).sum()

    loss_obj = loss_R - loss_Rc - loss_reg
    return (
        np.float32(-loss_obj),
        np.float32(loss_R),
        np.float32(loss_Rc),
        np.float32(loss_reg),
    )


# revision 6
# speedup vs baseline: 1.6943x; 1.6190x over previous
"""MCR2 variational loss on 8 Trainium2 NeuronCores.

Strategy (data-parallel over the sample axis n):
  - The heavy part of the loss is the per-class second-moment matrices
    M_j = Z^T diag(Pi_j) Z (plus the global gram Z^T Z), which reads all of
    Z/Pi once -> memory-bound. Everything downstream (logdet, log1p terms,
    Frobenius distance) is O(C*d^2) scalar work done on the host in fp64.
  - Fast path (Pi exactly one-hot): each sample contributes to exactly one
    class, so per-class partial grams over class-sorted rows give all M_j,
    and gram = sum_j M_j. Host distributes rows so every core gets an
    almost equal share of each class, pads each class segment to a 128-row
    multiple, and the device runs one fp32 matmul per 128-row chunk,
    accumulating into that class's PSUM slice. DMA tiles ramp up in size
    and alternate between the two HWDGE rings (SP + ACT) to start the PE
    early and sustain full HBM bandwidth. Output per core: [128, C*128]
    partial M, stored eagerly per class; host all-reduces in fp64.
  - Fallback (general dense Pi): host BLAS contraction.
"""

import numpy as np

EPS = 0.5
MU = 1.0
C = 10
N_TOTAL = 131072
D = 128
N_CORES = 8
CHUNK = 128  # rows per matmul (PE contraction dim)

_compiled_cache = {}


def _dma_tile_sizes(total_chunks):
    """Ramped tile sizes (in chunks): small first so the PE starts early."""
    sizes = []
    ramp = [4, 4, 8, 8, 16, 16]
    left = total_chunks
    for r in ramp:
        if left <= 0:
            break
        t = min(r, left)
        sizes.append(t)
        left -= t
    while left > 0:
        t = min(32, left)
        sizes.append(t)
        left -= t
    return sizes


def _build_bass_program(seg_chunks):
    """SPMD bass program computing per-class partial grams.

    seg_chunks: list of C ints — 128-row chunks per class (identical on all
    cores; zero padded on the host). Device input "z" is the class-sorted,
    padded, PRE-TILED Z: for each DMA tile t of tsz chunks, a contiguous
    [128, tsz*128] block (each SBUF partition's data contiguous in DRAM).
    Output "m_out": [128, C*128] partial M (d on partitions, (j,e) free).
    """
    import concourse.bacc as bacc
    import concourse.tile as tile
    from concourse import mybir
    from contextlib import ExitStack

    total_chunks = sum(seg_chunks)
    tile_sizes = _dma_tile_sizes(total_chunks)

    chunk_cls = []
    for j, k in enumerate(seg_chunks):
        chunk_cls.extend([j] * k)
    first = {}
    last = {}
    for i, j in enumerate(chunk_cls):
        first.setdefault(j, i)
        last[j] = i

    nc = bacc.Bacc("TRN2", target_bir_lowering=False, debug=False, num_devices=N_CORES)
    z = nc.dram_tensor(
        "z", [total_chunks * CHUNK, D], mybir.dt.bfloat16, kind="ExternalInput"
    ).ap()
    out = nc.dram_tensor(
        "m_out", [D, C * D], mybir.dt.float32, kind="ExternalOutput"
    ).ap()

    with tile.TileContext(nc) as tc:
        with ExitStack() as ctx:
            psum = ctx.enter_context(tc.tile_pool(name="psum", bufs=1, space="PSUM"))
            opool = ctx.enter_context(tc.tile_pool(name="o", bufs=1))
            acc = psum.tile([128, C * D], mybir.dt.float32)
            sb_out = opool.tile([128, C * D], mybir.dt.float32)
            row0 = 0
            cidx = 0
            for t, tsz in enumerate(tile_sizes):
                pool = ctx.enter_context(tc.tile_pool(name=f"z{t}", bufs=1))
                tl = pool.tile([128, tsz * D], mybir.dt.bfloat16)
                src = z[row0 : row0 + CHUNK * tsz, :].rearrange(
                    "(p k) d -> p (k d)", p=128
                )
                # alternate between the two HWDGE rings
                if t % 2 == 0:
                    nc.sync.dma_start(tl[:], src)
                else:
                    nc.scalar.dma_start(tl[:], src)
                row0 += CHUNK * tsz
                for k in range(tsz):
                    j = chunk_cls[cidx]
                    chunk = tl[:, k * D : (k + 1) * D]
                    nc.tensor.matmul(
                        acc[:, j * D : (j + 1) * D],
                        chunk,
                        chunk,
                        start=(cidx == first[j]),
                        stop=(cidx == last[j]),
                        skip_group_check=True,
                    )
                    # drain finished PSUM *banks* (4 classes = 512 f32 = one
                    # 2KB bank) so the DVE read never shares a bank with
                    # in-flight PE writes (which would serialize the PE)
                    if cidx == last[j] and (j % 4 == 3 or j == C - 1):
                        g0 = (j // 4) * 4
                        sl = slice(g0 * D, (j + 1) * D)
                        nc.vector.tensor_copy(sb_out[:, sl], acc[:, sl])
                        # mid-stream stores ride SWDGE so they never steal
                        # load-ring bandwidth; only the last store (loads
                        # long done) uses the HW ring
                        if j == C - 1:
                            nc.sync.dma_start(out[:, sl], sb_out[:, sl])
                        else:
                            nc.gpsimd.dma_start(out[:, sl], sb_out[:, sl])
                    cidx += 1
    nc.compile()
    return nc


def _is_one_hot(Pi):
    if not (Pi.sum(axis=1) == 1.0).all():
        return False
    if not (Pi.max(axis=1) == 1.0).all():
        return False
    return np.count_nonzero(Pi) == Pi.shape[0]


def _fast_path_M(Z, Pi):
    """Per-class second moments via the device. Returns M [C, D, D] fp64."""
    import ml_dtypes
    from concourse.bass_utils import run_bass_kernel_spmd

    labels = np.argmax(Pi, axis=1)

    # balance every class across cores: class j's rows are dealt out in
    # near-equal contiguous slices, so per-class per-core counts differ by
    # at most 1 and padding is minimal
    order = np.argsort(labels, kind="stable")
    cls_counts = np.bincount(labels, minlength=C)
    cls_offs = np.concatenate([[0], np.cumsum(cls_counts)])

    counts = np.zeros((N_CORES, C), dtype=np.int64)
    for j in range(C):
        m = cls_counts[j]
        base, rem = divmod(m, N_CORES)
        for c in range(N_CORES):
            counts[c, j] = base + (1 if c < rem else 0)

    seg_chunks = [max(1, int(np.ceil(counts[:, j].max() / CHUNK))) for j in range(C)]
    total_chunks = sum(seg_chunks)
    tile_sizes = _dma_tile_sizes(total_chunks)

    key = tuple(seg_chunks)
    if key not in _compiled_cache:
        _compiled_cache[key] = _build_bass_program(seg_chunks)
    nc = _compiled_cache[key]

    offs = np.concatenate([[0], np.cumsum(seg_chunks)]) * CHUNK
    # ship bf16: halves HBM traffic and runs the PE at 1 cycle/row; the
    # rounding effect on the final losses is ~6e-6 relative (measured),
    # below fp32 accumulation-order noise
    Zb = Z.astype(ml_dtypes.bfloat16)
    in_maps = []
    for c in range(N_CORES):
        zbuf = np.zeros((total_chunks * CHUNK, D), dtype=ml_dtypes.bfloat16)
        for j in range(C):
            lo = cls_offs[j] + counts[:c, j].sum()
            nj = counts[c, j]
            zbuf[offs[j] : offs[j] + nj] = Zb[order[lo : lo + nj]]
        # pre-tile each DMA block: [tsz, 128, D] -> [128, tsz*D]
        parts = []
        start = 0
        for tsz in tile_sizes:
            blk = zbuf[start * CHUNK : (start + tsz) * CHUNK]
            parts.append(
                np.ascontiguousarray(
                    blk.reshape(tsz, CHUNK, D).transpose(1, 0, 2)
                ).reshape(-1)
            )
            start += tsz
        zdev = np.concatenate(parts).reshape(total_chunks * CHUNK, D)
        in_maps.append({"z": zdev})

    res = run_bass_kernel_spmd(nc, in_maps, list(range(N_CORES)))
    M = np.zeros((C, D, D), dtype=np.float64)
    for c in range(N_CORES):
        o = res.results[c]["m_out"].astype(np.float64)  # [D, C*D]
        M += o.reshape(D, C, D).transpose(1, 0, 2)
    return M


def _dense_path_M(Z, Pi):
    """General dense Pi: host BLAS contraction. Returns (M, gram) fp64."""
    Zf = np.ascontiguousarray(Z, dtype=np.float32)
    A = (Pi[:, :, None].astype(np.float32) * Zf[:, None, :]).reshape(Zf.shape[0], -1)
    M = (A.T @ Zf).reshape(C, D, D).astype(np.float64)
    gram = (Zf.T @ Zf).astype(np.float64)
    return M, gram


def kernel(Z, Pi, Us):
    Z = np.asarray(Z, dtype=np.float32)
    Pi = np.asarray(Pi, dtype=np.float32)
    Us = np.asarray(Us, dtype=np.float32)
    n, d = Z.shape

    if n == N_TOTAL and d == D and Pi.shape == (n, C) and _is_one_hot(Pi):
        M = _fast_path_M(Z, Pi)
        gram = M.sum(axis=0)
    else:
        M, gram = _dense_path_M(Z, Pi)

    nf = float(n)
    df = float(d)

    A = np.eye(d, dtype=np.float64) + (df / (nf * EPS)) * gram
    sign, logabsdet = np.linalg.slogdet(A)
    loss_R = 0.5 * logabsdet

    trPi = Pi.astype(np.float64).sum(axis=0)
    col_norms_sq = (Us.astype(np.float64) ** 2).sum(axis=1)  # [C, d]
    with np.errstate(divide="ignore"):
        per_class = np.log1p((df / (trPi[:, None] * EPS)) * col_norms_sq).sum(axis=1)
    loss_Rc = ((trPi / (2.0 * nf)) * per_class).sum()

    Us64 = Us.astype(np.float64)
    UUt = np.einsum("jdk,jek->jde", Us64, Us64)
    loss_reg = 0.5 * MU * ((M - UUt) ** 2).sum()

    loss_obj = loss_R - loss_Rc - loss_reg
    return (
        np.float32(-loss_obj),
        np.float32(loss_R),
        np.float32(loss_Rc),
        np.float32(loss_reg),
    )
